# revision 1
# baseline (speedup 1.0000x reference)
"""Trainium2 Bass kernel for nn_MultiHeadAttention_19396072309379.

Module math (per reference): all H=8 heads identical; V projected from `key`;
causal mask; softmax; concat of identical heads @ Wo  ==  o @ (sum of Wo row
blocks).  Computed as single-head attention with a reduced Wo.

Sharding: 8 cores = 4 batches x 2 "parity" halves.  Each core owns 8 of the 16
query blocks (128 rows each) of one batch, paired {i, 15-i} so causal work is
balanced (68 block-pairs per core).  Both parities run the SAME program: the
attention loop uses unified per-key-block suffix widths (max over parities) and
a per-core mask input resolves the diagonal/extra-block difference as data.

On-chip layout is fully transposed ("T" = [feature, seq]): scoresT[ks, qs] =
kT_proj_blk.T @ qT_proj; exp via ACT (scale=1/sqrt(DK) fused); P stays
transposed so PV needs no P transpose: oT = v1.T @ expT where v1 = [v | 1] --
the ones column accumulates the softmax denominators for free (row 64).
Normalization is folded in after a K=1 broadcast matmul of the sums row.
All matmuls run fp32r (full-rate fp32 mode, ~1.6e-4 rel err).

Pipelining: key/value flow in 512-column chunks -- DMA chunk n -> k/v proj ->
v transpose -> attention key-blocks 4n..4n+3, so compute hides under the input
DMA.  The PV accumulator is split into two PSUM banks (query blocks 0-3 / 4-7);
the first half finalizes at j=7, so its normalization + output projection +
store overlap the second half of the attention loop.
"""

import numpy as np

B, S, D, H, DK, DV = 4, 2048, 512, 8, 64, 64
NB = S // 128  # 16 key/query blocks per batch
QB = 8  # query blocks per core
SQ = QB * 128  # 1024 query rows per core
N_CORES = 8

# per-parity query block sets (pairs {i, 15-i} -> equal causal work 68)
BLOCKS = {
    0: [0, 2, 4, 6, 9, 11, 13, 15],
    1: [1, 3, 5, 7, 8, 10, 12, 14],
}
# unified suffix width (in 128-blocks) for key-block j = max over parities of
# count of local query blocks with global index >= j
WIDTHS = [
    max(sum(1 for g in BLOCKS[p] if g >= j) for p in (0, 1)) for j in range(NB)
]


def _build(reps=1):
    import concourse.mybir as mybir
    import concourse.tile as tile
    from concourse import bacc

    F32 = mybir.dt.float32
    F32R = mybir.dt.float32r
    U8 = mybir.dt.uint8
    AF = mybir.ActivationFunctionType

    nc = bacc.Bacc("TRN2", target_bir_lowering=False, debug=False, num_devices=N_CORES)
    F16 = mybir.dt.float16
    d_qT = nc.dram_tensor("qT", [D, SQ], F16, kind="ExternalInput").ap()
    d_kT = nc.dram_tensor("kT", [D, S], F16, kind="ExternalInput").ap()
    d_wqkv = nc.dram_tensor("wqkv", [D, 6 * DK], F16, kind="ExternalInput").ap()
    d_b2 = nc.dram_tensor("b2", [128, 2], F32, kind="ExternalInput").ap()
    d_c64 = nc.dram_tensor("c64", [DV, D + 2 + 2 * DV], F32, kind="ExternalInput").ap()
    d_bm = nc.dram_tensor("bm", [128, NB + 2, 128], U8, kind="ExternalInput").ap()
    d_out = nc.dram_tensor("out", [SQ, D], F16, kind="ExternalOutput").ap()

    for _ in range(reps):
        _emit_body(
            nc, tile, mybir, F32, F32R, AF,
            d_qT, d_kT, d_wqkv, d_c64, d_bm, d_b2, d_out,
        )
    nc.compile()
    return nc


def _emit_body(nc, tile, mybir, F32, F32R, AF,
               d_qT, d_kT, d_wqkv, d_c64, d_bm, d_b2, d_out):
    F16 = mybir.dt.float16
    with (
        tile.TileContext(nc) as tc,
        nc.allow_low_precision(reason="fp32r attention kernel"),
    ):
        with (
            tc.tile_pool(name="const", bufs=1) as cpool,
            tc.tile_pool(name="acts", bufs=1) as apool,
            tc.tile_pool(name="work", bufs=3) as wpool,
            tc.tile_pool(name="psmall", bufs=2, space="PSUM") as psm,
            tc.tile_pool(name="pscore", bufs=3, space="PSUM") as psc,
            tc.tile_pool(name="pacc", bufs=1, space="PSUM") as pacc,
        ):
            # ---- weights first (small), then query chunk 0: the q-proj
            # chain gates everything and HWDGE transfers serialize in
            # emission order ----
            # wqkv_t[:, c, 0:64]=Wq, 64:128=Wk, 128:192=Wv (d-chunk c)
            wqkv_t = cpool.tile([128, 4, 6 * DK], F16)
            nc.sync.dma_start(
                wqkv_t[:], d_wqkv.rearrange("(c p) k -> p c k", p=128)
            )
            b2_t = cpool.tile([128, 2], F32)
            nc.sync.dma_start(b2_t[:], d_b2[:])
            qT_act = apool.tile([128, 4, SQ], F16)
            qT_r0 = d_qT.rearrange("(c p) s -> p c s", p=128)
            nc.sync.dma_start(qT_act[:, :, 0:512], qT_r0[:, :, 0:512])
            # uint8 mask (with extra all-ones plane), cast to f32 in-flight
            # on the SWDGE ring, parallel to HWDGE
            bm_t = cpool.tile([128, NB + 2, 128], F32)
            nc.gpsimd.dma_start(bm_t[:], d_bm[:])
            # c64_t: [64, 0:512]=Wo_r, 512=bq, 513=bk, 514:578=id, 578:642=1s
            c64_t = cpool.tile([DV, D + 2 + 2 * DV], F32R)
            nc.scalar.dma_start(c64_t[:], d_c64.bitcast(F32R))
            wq_t = wqkv_t[:, :, 0 : 2 * DK]          # Wq|Wq (M=128)
            wk_t = wqkv_t[:, :, 2 * DK : 4 * DK]     # Wk|Wk (M=128)
            wv_t = wqkv_t[:, :, 4 * DK : 6 * DK]  # Wv|Wv (M=128)
            wo_t = c64_t[:, 0:D]
            bq_t = c64_t[:, D : D + 1].bitcast(F32)
            bk_t = c64_t[:, D + 1 : D + 2].bitcast(F32)
            id_t = c64_t[:, D + 2 : D + 2 + DV].bitcast(F32)
            ones_row = c64_t[0:1, D + 2 + DV : D + 2 + 2 * DV]

            # ---- PE p-state warm-up: ~6us of zero matmuls during the
            # input-DMA window (tensor engine needs ~3us of continuous busy
            # to reach full clock) ----
            warm = cpool.tile([128, 512], F16)
            nc.gpsimd.memset(warm[:, 0:64], 0.0)
            wps = psm.tile([128, 512], F32, tag="pp", name="wps")
            for _ in range(45):
                nc.tensor.matmul(
                    wps[0:64, 0:64], warm[:, 0:64], warm[:, 0:64],
                    start=True, stop=True, skip_group_check=True,
                )
            nc.gpsimd.memset(warm[:, 64:512], 0.0)
            for _ in range(8):
                nc.tensor.matmul(
                    wps[0:64, 0:512], warm[:, 0:64], warm[:, 0:512],
                    start=True, stop=True, skip_group_check=True,
                )

            # ---- persistent SBUF tensors ----
            kT_act = apool.tile([128, 4, S], F16)
            qT_proj = apool.tile([128, SQ], F32R)
            kT_proj = apool.tile([128, S], F32R)
            vT_proj = apool.tile([128, S], F32)
            v1 = apool.tile([128, NB, DV + 1], F32R)
            # accumulator regions: [lo, hi, last_j]; each its own PSUM bank
            PO_R = [(0, 512, 7), (512, 768, 11), (768, 1024, 15)]
            po = [
                pacc.tile([DV + 1, hi - lo], F32, name=f"po{q}")
                for q, (lo, hi, _) in enumerate(PO_R)
            ]

            qT_r = d_qT.rearrange("(c p) s -> p c s", p=128)
            kT_r = d_kT.rearrange("(c p) s -> p c s", p=128)

            # ---- remaining input DMAs up front (few, large) ----
            nc.sync.dma_start(qT_act[:, :, 512:1024], qT_r[:, :, 512:1024])
            for n in range(S // 512):
                sl = slice(512 * n, 512 * (n + 1))
                nc.sync.dma_start(kT_act[:, :, sl], kT_r[:, :, sl])
            # ones column of every v1 block in one shot (from the mask's
            # all-ones plane)
            nc.vector.tensor_copy(
                v1[:, :, DV : DV + 1].rearrange("p j o -> p (j o)"),
                bm_t[:, NB, 0:NB].bitcast(F32R),
            )

            def qproj(n):
                sl = slice(512 * n, 512 * (n + 1))
                ps = psm.tile([128, 512], F32, tag="pp", name="psq")
                for c in range(4):
                    nc.tensor.matmul(
                        ps[:], wq_t[:, c, :], qT_act[:, c, sl],
                        start=(c == 0), stop=(c == 3),
                    )
                nc.vector.tensor_scalar_add(qT_proj[:, sl], ps[:], b2_t[:, 0:1])

            def kvproj(n):
                sl = slice(512 * n, 512 * (n + 1))
                ps = psm.tile([128, 512], F32, tag="pp", name="psk")
                for c in range(4):
                    nc.tensor.matmul(
                        ps[:], wk_t[:, c, :], kT_act[:, c, sl],
                        start=(c == 0), stop=(c == 3),
                    )
                nc.vector.tensor_scalar_add(kT_proj[:, sl], ps[:], b2_t[:, 1:2])
                ps = psm.tile([128, 512], F32, tag="pp", name="psv")
                for c in range(4):
                    nc.tensor.matmul(
                        ps[:], wv_t[:, c, :], kT_act[:, c, sl],
                        start=(c == 0), stop=(c == 3),
                    )
                nc.vector.tensor_copy(vT_proj[:, sl], ps[:])

            def transpose_v(j):
                # v1[j][:, 0:DV] = v block via PE transpose of vT columns;
                # consecutive blocks alternate PE row halves (concurrent)
                rh = DV * (j % 2)
                pt = psm.tile([128, DV], F32, tag="pp", name="pt")
                nc.tensor.transpose(
                    pt[:],
                    vT_proj[rh : rh + DV, 128 * j : 128 * (j + 1)],
                    bm_t[rh : rh + DV, NB + 1, 0:DV],
                    tile_position=(rh, 0),
                )
                nc.vector.tensor_copy(v1[:, j, 0:DV], pt[:])

            exps = {}

            def scores_exp(j):
                wblk = WIDTHS[j]
                c0 = 128 * (QB - wblk)
                cols = 128 * wblk
                expT = wpool.tile([128, 1024], F32R, tag="expT", bufs=6)
                rh = 64 * (j % 2)  # alternate PE row halves: consecutive
                # key-blocks' K=64 score matmuls run concurrently
                for m in range((cols + 511) // 512):
                    nsz = min(512, cols - 512 * m)
                    ps_s = psc.tile([128, 512], F32, tag="pss")
                    nc.tensor.matmul(
                        ps_s[:, 0:nsz],
                        kT_proj[rh : rh + DK, 128 * j : 128 * (j + 1)],
                        qT_proj[rh : rh + DK, c0 + 512 * m : c0 + 512 * m + nsz],
                        start=True, stop=True,
                        tile_position=(rh, 0),
                    )
                    nc.scalar.activation(
                        expT[:, 512 * m : 512 * m + nsz], ps_s[:, 0:nsz],
                        AF.Exp, bias=0.0, scale=0.125,
                    )
                nc.vector.tensor_mul(
                    expT[:, 0:128], expT[:, 0:128],
                    bm_t[:, j, :].bitcast(F32R),
                )
                exps[j] = expT

            def emit_pv(j, part):
                """part 0: columns past the masked first block (independent of
                the mask op); part 1: the masked first 128 columns; part 2:
                the full range (used for j == 0, whose start=True clears the
                whole PSUM bank and therefore must be a single first write)."""
                wblk = WIDTHS[j]
                c0 = 128 * (QB - wblk)
                lo0, hi0 = (c0 + 128, SQ) if part == 0 else (c0, c0 + 128)
                if part == 2:
                    lo0, hi0 = c0, SQ
                for q, (rlo, rhi, lastj) in enumerate(PO_R):
                    lo = max(lo0, rlo)
                    hi = min(hi0, rhi)
                    if lo >= hi:
                        continue
                    nc.tensor.matmul(
                        po[q][:, lo - rlo : hi - rlo],
                        v1[:, j, :],
                        exps[j][:, lo - c0 : hi - c0],
                        start=(j == 0),
                        stop=(j == lastj and part != 0),
                        skip_group_check=True,
                    )

            # staged epilogue for accumulator region q
            ep_state = {}

            def ep_a(q):
                p = po[q]
                w = PO_R[q][1] - PO_R[q][0]
                oT_s = apool.tile([DV, 512], F32R, name=f"oTs{q}", tag=f"oTs{q}")
                nc.scalar.copy(oT_s[:, 0:w], p[0:DV, :])
                srow = apool.tile([1, 512], F32R, name=f"srow{q}", tag=f"srow{q}")
                nc.vector.tensor_copy(srow[:, 0:w], p[DV : DV + 1, :])
                pb = psm.tile([DV, 512], F32, tag="pp", name=f"pb{q}")
                nc.tensor.matmul(
                    pb[:, 0:w], ones_row, srow[:, 0:w],
                    start=True, stop=True,
                )
                ep_state[q] = (oT_s, pb)

            def ep_b(q):
                oT_s, pb = ep_state[q]
                w = PO_R[q][1] - PO_R[q][0]
                rec = apool.tile([DV, 512], F32R, name=f"rec{q}", tag=f"rec{q}")
                nc.vector.reciprocal(rec[:, 0:w], pb[:, 0:w])
                oT_n = apool.tile([DV, 512], F32R, name=f"oTn{q}", tag=f"oTn{q}")
                nc.vector.tensor_mul(oT_n[:, 0:w], oT_s[:, 0:w], rec[:, 0:w])
                ep_state[q] = oT_n

            def ep_c(q, i):
                oT_n = ep_state[q]
                pf = psm.tile([128, D], F32, tag="pp", name=f"pf{q}")
                nc.tensor.matmul(
                    pf[:], oT_n[:, 128 * i : 128 * (i + 1)], wo_t[:],
                    start=True, stop=True,
                )
                osb = wpool.tile([128, D], F16, tag="osb", name=f"osb{q}")
                if i % 2 == 0:
                    nc.vector.tensor_copy(osb[:], pf[:])
                else:
                    nc.scalar.copy(osb[:], pf[:])
                qb = PO_R[q][0] // 128 + i
                eng = nc.sync if i % 2 == 0 else nc.scalar
                eng.dma_start(d_out[128 * qb : 128 * (qb + 1), :], osb[:])

            # ---- emission schedule: projections lead their consumer group;
            # PV trails exp (bulk by 1, masked block by 2); epilogue(0) is
            # spread across iterations 9..14 so the in-order PE never camps
            # behind its serial ACT->DVE chain. ----
            EP0 = {9: lambda: ep_a(0), 10: lambda: ep_b(0),
                   11: lambda: ep_c(0, 0), 12: lambda: ep_c(0, 1),
                   13: lambda: (ep_c(0, 2), ep_a(1)),
                   14: lambda: (ep_c(0, 3), ep_b(1)),
                   15: lambda: ep_c(1, 0)}
            qproj(0)
            qproj(1)
            kvproj(0)
            kvproj(1)
            for j in range(4):
                transpose_v(j)
            for j in range(NB):
                if j == 2:
                    for jj in range(4, 8):
                        transpose_v(jj)
                if j == 4:
                    kvproj(2)
                if j == 6:
                    for jj in range(8, 12):
                        transpose_v(jj)
                if j == 8:
                    kvproj(3)
                if j == 10:
                    for jj in range(12, NB):
                        transpose_v(jj)
                scores_exp(j)
                if j == 1:
                    emit_pv(0, 2)  # j=0 unsplit: single start=True per bank
                elif j >= 2:
                    emit_pv(j - 1, 0)
                if j >= 3:
                    emit_pv(j - 2, 1)
                if j in EP0:
                    EP0[j]()
            emit_pv(NB - 1, 0)
            emit_pv(NB - 2, 1)
            ep_c(1, 1)
            emit_pv(NB - 1, 1)
            ep_a(2)
            ep_b(2)
            ep_c(2, 0)
            ep_c(2, 1)


_NC_CACHE = None


def _get_nc():
    global _NC_CACHE
    if _NC_CACHE is None:
        _NC_CACHE = _build()
    return _NC_CACHE


def make_in_maps(query, key, Wq, bq, Wk, bk, Wv, bv, Wo, bo):
    query = np.asarray(query, dtype=np.float32)
    key = np.asarray(key, dtype=np.float32)
    Wq = np.asarray(Wq, dtype=np.float32)
    Wk = np.asarray(Wk, dtype=np.float32)
    Wv = np.asarray(Wv, dtype=np.float32)
    Wo = np.asarray(Wo, dtype=np.float32)
    bq = np.asarray(bq, dtype=np.float32)
    bk = np.asarray(bk, dtype=np.float32)

    wo_r = np.ascontiguousarray(Wo.reshape(H, DV, D).sum(axis=0))  # [DV, D]
    wqkv = np.concatenate([Wq, Wq, Wk, Wk, Wv, Wv], axis=1).astype(np.float16)
    b2 = np.stack(
        [np.concatenate([bq, bq]), np.concatenate([bk, bk])], axis=1
    ).astype(np.float32)  # [128, 2]
    c64 = np.concatenate(
        [wo_r, bq.reshape(DV, 1), bk.reshape(DV, 1),
         np.eye(DV, dtype=np.float32), np.ones((DV, DV), np.float32)],
        axis=1,
    )  # [64, 642]
    tri = np.triu(np.ones((128, 128), np.uint8))  # valid: ks <= qs

    in_maps = []
    for c in range(N_CORES):
        b, p = divmod(c, 2)
        blocks = BLOCKS[p]
        rows = np.concatenate(
            [np.arange(128 * g, 128 * (g + 1)) for g in blocks]
        )
        qT = np.ascontiguousarray(query[b][rows].T).astype(np.float16)
        kT = np.ascontiguousarray(key[b].T).astype(np.float16)
        bm = np.empty((NB + 2, 128, 128), np.uint8)
        bm[NB] = 1
        bm[NB + 1] = 0
        bm[NB + 1, 0:64, 0:64] = np.eye(64, dtype=np.uint8)
        bm[NB + 1, 64:128, 0:64] = np.eye(64, dtype=np.uint8)
        for j in range(NB):
            g = blocks[QB - WIDTHS[j]]
            if g == j:
                bm[j] = tri
            elif g > j:
                bm[j] = 1
            else:
                bm[j] = 0
        bm = np.ascontiguousarray(bm.transpose(1, 0, 2))  # [128, NB, 128]
        in_maps.append(
            {"qT": qT, "kT": kT, "wqkv": wqkv, "c64": c64, "bm": bm, "b2": b2}
        )
    return in_maps


def gather_output(results, bias_term):
    """results: list of per-core {'out': [SQ, D]}; adds host-folded bias."""
    out = np.empty((B, S, D), np.float32)
    for c in range(N_CORES):
        b, p = divmod(c, 2)
        blocks = BLOCKS[p]
        co = np.asarray(results[c]["out"], dtype=np.float32)
        for bp, g in enumerate(blocks):
            out[b, 128 * g : 128 * (g + 1), :] = co[128 * bp : 128 * (bp + 1), :]
    out += bias_term
    return out


def kernel(query, key, value, Wq, bq, Wk, bk, Wv, bv, Wo, bo):
    from concourse import bass_utils

    nc = _get_nc()
    in_maps = make_in_maps(query, key, Wq, bq, Wk, bk, Wv, bv, Wo, bo)
    res = bass_utils.run_bass_kernel_spmd(
        nc, in_maps, core_ids=list(range(N_CORES))
    )
    Wo = np.asarray(Wo, dtype=np.float32)
    wo_r = Wo.reshape(H, DV, D).sum(axis=0)
    bias_term = np.asarray(bv, np.float32) @ wo_r + np.asarray(bo, np.float32)
    return gather_output(res.results, bias_term.astype(np.float32))



# revision 33
# speedup vs baseline: 1.3326x; 1.3326x over previous
"""Trainium2 Bass kernel for nn_MultiHeadAttention_19396072309379.

Module math (per reference): all H=8 heads identical; V projected from `key`;
causal mask; softmax; concat of identical heads @ Wo  ==  o @ (sum of Wo row
blocks).  Computed as single-head attention with a reduced Wo.

Key simplifications vs a naive lowering:
  * bq/bk shift scores by a per-query constant -> softmax-invariant -> dropped.
    bv contributes bv @ Wo_r to every output row (attn rows sum to 1) -> folded
    into the host-side output bias.  No bias ops on-chip at all.
  * Causal/parity masks multiply the first (partial) 128-col block of each
    key-block's exp'd scores.  The per-j planes {tri | ones | zeros} are
    synthesized on-chip as tri * b_j + a_j from per-core selector scalars,
    which resolves the parity-dependent mask choice with a shared program.
  * V is computed directly row-major (out partitions = tokens), so no PE
    transpose / vT staging is needed; an extra ones-column in v1 accumulates
    softmax denominators for free inside the PV matmul.
  * Softmax normalization is deferred past the output projection: out rows
    are scaled by 1/denom in the PSUM->SBUF copy (DVE tensor_scalar / ACT
    activation-scale).  Denominators are transposed into per-partition layout
    with tiny PE row transposes.

Sharding: 8 cores = 4 batches x 2 "parity" halves.  Each core owns 8 of the
16 query blocks (128 rows each) of one batch, paired {i, 15-i} so causal work
is balanced.  Both parities run the SAME program with unified per-key-block
suffix widths (max over parities).

On-chip layout is fully transposed ("T" = [feature, seq]); all attention
operands are f16 (full-rate PE for any tile size, 2x/4x DVE modes), PSUM
accumulation f32.  K/V flow in 512-column chunks overlapped with input DMA.
"""

import numpy as np

B, S, D, H, DK, DV = 4, 2048, 512, 8, 64, 64
NB = S // 128  # 16 key/query blocks per batch
QB = 8  # query blocks per core
SQ = QB * 128  # 1024 query rows per core
N_CORES = 8

# per-parity query block sets (pairs {i, 15-i} -> equal causal work 68)
BLOCKS = {
    0: [0, 2, 4, 6, 9, 11, 13, 15],
    1: [1, 3, 5, 7, 8, 10, 12, 14],
}
# unified suffix width (in 128-blocks) for key-block j = max over parities of
# count of local query blocks with global index >= j
WIDTHS = [
    max(sum(1 for g in BLOCKS[p] if g >= j) for p in (0, 1)) for j in range(NB)
]

# PV accumulator regions: (col_lo, col_hi, psum_tile_idx, tile_col_off, last_j)
PO_R = [(0, 512, 0, 0, 7), (512, 896, 1, 0, 13), (896, 1024, 1, 384, 15)]
# output chunks per region (global 128-col chunk indices)
R_CHUNKS = [(0, 1, 2, 3), (4, 5, 6), (7,)]

# cst (f16) column layout
C_WQKV = 0          # 4 chunks x [Wq|Wq|Wk|Wk|Wv|Wv] (384 each)
C_TRI = 1536        # causal tri plane tri[ks, qs] = (qs >= ks)
C_NCOL = 1664

N_WARM = 8  # PE p-state warm-up matmuls (512 cols each)


def _build(reps=1, dbg=False):
    import concourse.mybir as mybir
    import concourse.tile as tile
    from concourse import bacc

    F32 = mybir.dt.float32
    F16 = mybir.dt.float16

    nc = bacc.Bacc("TRN2", target_bir_lowering=False, debug=False, num_devices=N_CORES)
    d_qT = nc.dram_tensor("qT", [D, SQ], F16, kind="ExternalInput").ap()
    d_kT = nc.dram_tensor("kT", [D, S], F16, kind="ExternalInput").ap()
    d_cst = nc.dram_tensor("cst", [128, C_NCOL], F16, kind="ExternalInput").ap()
    d_wo = nc.dram_tensor("wo", [DV, D], F16, kind="ExternalInput").ap()
    d_msk = nc.dram_tensor("msk", [128, 2 * NB + 1], F32, kind="ExternalInput").ap()
    d_out = nc.dram_tensor("out", [SQ, D], F16, kind="ExternalOutput").ap()

    for _ in range(reps):
        _emit_body(nc, tile, mybir, d_qT, d_kT, d_cst, d_wo, d_msk, d_out)
    nc.compile()
    return nc


def _emit_body(nc, tile, mybir, d_qT, d_kT, d_cst, d_wo, d_msk, d_out):
    F32 = mybir.dt.float32
    F16 = mybir.dt.float16
    AF = mybir.ActivationFunctionType
    ALU = mybir.AluOpType

    with (
        tile.TileContext(nc) as tc,
        nc.allow_low_precision(reason="f16 attention kernel"),
    ):
        with (
            tc.tile_pool(name="const", bufs=1) as cpool,
            tc.tile_pool(name="acts", bufs=1) as apool,
            tc.tile_pool(name="work", bufs=3) as wpool,
            tc.tile_pool(name="outb", bufs=8) as opool,
            tc.tile_pool(name="psmisc", bufs=2, space="PSUM") as psm,
            tc.tile_pool(name="pscore", bufs=2, space="PSUM") as psc,
            tc.tile_pool(name="pacc", bufs=1, space="PSUM") as pacc,
        ):
            # ---- PE warm-up starts immediately (DVE memset -> zero matmuls)
            # so the modeled p-state ramp completes during the input DMAs ----
            warm = cpool.tile([128, 512], F16)
            nc.vector.memset(warm[:], 0.0)
            wps = psm.tile([128, 512], F32, tag="pp", name="wps")
            for _ in range(N_WARM):
                nc.tensor.matmul(
                    wps[:], warm[:, 0:128], warm[:],
                    start=True, stop=True, skip_group_check=True,
                )

            # ---- input DMAs (order = transfer order on the shared DMA rsrc):
            # consts, then q (gates qproj), then k chunks ----
            cst_t = cpool.tile([128, C_NCOL], F16)
            nc.sync.dma_start(cst_t[:], d_cst[:])
            qT_act = apool.tile([128, 4, SQ], F16)
            qT_r = d_qT.rearrange("(c p) s -> p c s", p=128)
            kT_r = d_kT.rearrange("(c p) s -> p c s", p=128)
            nc.sync.dma_start(qT_act[:, :, 0:512], qT_r[:, :, 0:512])
            msk_t = cpool.tile([128, 2 * NB + 1], F32)
            nc.sync.dma_start(msk_t[:], d_msk[:])
            nc.sync.dma_start(qT_act[:, :, 512:1024], qT_r[:, :, 512:1024])
            kT_act = apool.tile([128, 4, S], F16)
            nc.sync.dma_start(kT_act[:, :, 0:512], kT_r[:, :, 0:512])
            wo_t = cpool.tile([DV, D], F16)
            nc.sync.dma_start(wo_t[:], d_wo[:])
            for n in range(1, 4):
                sl = slice(512 * n, 512 * (n + 1))
                nc.sync.dma_start(kT_act[:, :, sl], kT_r[:, :, sl])

            id1 = msk_t[0:1, 2 * NB : 2 * NB + 1]
            # per-j mask planes {tri | ones | zeros} synthesized as
            # tri * b_j + a_j with per-core selector scalars; runs on idle
            # DVE time during the input-DMA window.
            tri_t = cst_t[:, C_TRI : C_TRI + 128]
            bm16 = apool.tile([128, NB, 128], F16)
            for j in range(NB):
                nc.vector.tensor_scalar(
                    bm16[:, j, :], tri_t[:],
                    msk_t[:, j : j + 1], msk_t[:, NB + j : NB + j + 1],
                    ALU.mult, ALU.add,
                )

            # ---- persistent SBUF tensors ----
            qT_proj = apool.tile([128, SQ], F16)
            kT_proj = apool.tile([128, S], F16)
            v1 = apool.tile([128, NB, DV + 1], F16)
            nc.vector.memset(v1[:, :, DV : DV + 1], 1.0)
            oT = apool.tile([DV, SQ], F16)
            srow = apool.tile([1, SQ], F32)
            recipT = apool.tile([128, QB], F32)
            po = [
                pacc.tile([DV + 1, 512], F32, name="po0"),
                pacc.tile([DV + 1, 512], F32, name="po12"),
            ]

            def qproj(n):
                sl = slice(512 * n, 512 * (n + 1))
                ps = psm.tile([128, 512], F32, tag="pp", name="psq")
                for c in range(4):
                    w = cst_t[:, 384 * c : 384 * c + 128]
                    nc.tensor.matmul(
                        ps[:], w, qT_act[:, c, sl], start=(c == 0), stop=(c == 3)
                    )
                nc.scalar.copy(qT_proj[:, sl], ps[:])

            def kvproj(n):
                sl = slice(512 * n, 512 * (n + 1))
                ps = psm.tile([128, 512], F32, tag="pp", name="psk")
                for c in range(4):
                    w = cst_t[:, 384 * c + 128 : 384 * c + 256]
                    nc.tensor.matmul(
                        ps[:], w, kT_act[:, c, sl], start=(c == 0), stop=(c == 3)
                    )
                nc.vector.tensor_copy(kT_proj[:, sl], ps[:])
                # v computed directly row-major: out partitions = tokens, so
                # no PE transpose / vT staging is needed.  4 token-blocks per
                # chunk; groups must stay contiguous (start=True marks the
                # tile's whole psum bank pending-zero).
                ps = psm.tile([128, 512], F32, tag="pp", name="psv")
                for jb in range(4):
                    tok = slice(512 * n + 128 * jb, 512 * n + 128 * jb + 128)
                    for c in range(4):
                        w = cst_t[:, 384 * c + 256 : 384 * c + 384]
                        nc.tensor.matmul(
                            ps[:, 64 * jb : 64 * jb + 64],
                            kT_act[:, c, tok], w[:, 0:DV],
                            start=(c == 0), stop=(c == 3),
                            skip_group_check=True,
                        )
                nc.vector.tensor_copy(v1[:, 4 * n : 4 * n + 4, 0:DV], ps[:, 0:256])

            exps = {}

            def scores(*js):
                # one psc tile + one exp activation shared by the (narrow)
                # key-blocks js; per-j mask multiplies on each first block
                ps_s = psc.tile([128, 1024], F32, tag="pss")
                expT = wpool.tile([128, 1024], F16, tag="expT", bufs=4)
                off = 0
                offs = []
                for j in js:
                    wblk = WIDTHS[j]
                    c0 = 128 * (QB - wblk)
                    cols = 128 * wblk
                    rh = 64 * (j % 2)
                    done = 0
                    while done < cols:
                        # matmul writes must not cross a psum bank boundary
                        nsz = min(512 - (off + done) % 512, cols - done)
                        nc.tensor.matmul(
                            ps_s[:, off + done : off + done + nsz],
                            kT_proj[rh : rh + DK, 128 * j : 128 * (j + 1)],
                            qT_proj[rh : rh + DK, c0 + done : c0 + done + nsz],
                            start=True, stop=True,
                            tile_position=(rh, 0),
                        )
                        done += nsz
                    offs.append((j, off, cols))
                    off += cols
                nc.scalar.activation(
                    expT[:, 0:off], ps_s[:, 0:off], AF.Exp, bias=0.0, scale=0.125
                )
                for j, o, cols in offs:
                    nc.vector.tensor_mul(
                        expT[:, o : o + 128], expT[:, o : o + 128], bm16[:, j, :]
                    )
                    exps[j] = expT[:, o : o + cols]

            def emit_pv(j, masked):
                # masked=False: bulk columns (depend on exp only);
                # masked=True: the first 128-col block (awaits the mask mul).
                # start=True marks the tile's WHOLE 2KB psum bank pending-
                # zero, so only the first write of each bank carries it; the
                # bank-sharing region 2 and the masked block are zero-filled
                # by the pending state instead.
                wblk = WIDTHS[j]
                c0 = 128 * (QB - wblk)
                for rlo, rhi, ti, toff, lastj in PO_R:
                    lo = max(c0, rlo)
                    if lo >= rhi:
                        continue
                    if lo == c0:
                        a, b = (c0, c0 + 128) if masked else (c0 + 128, rhi)
                    else:
                        if masked:
                            continue
                        a, b = lo, rhi
                    if a >= b:
                        continue
                    nc.tensor.matmul(
                        po[ti][:, toff + a - rlo : toff + b - rlo],
                        v1[:, j, :],
                        exps[j][:, a - c0 : b - c0],
                        start=(j == 0 and toff == 0 and not masked),
                        stop=(j == lastj),
                        skip_group_check=True,
                    )

            def pv(j):
                emit_pv(j, False)
                emit_pv(j, True)

            def ep_copy(q):
                rlo, rhi, ti, toff, _ = PO_R[q]
                w = rhi - rlo
                nc.scalar.copy(oT[:, rlo:rhi], po[ti][0:DV, toff : toff + w])
                nc.vector.tensor_copy(
                    srow[:, rlo:rhi], po[ti][DV : DV + 1, toff : toff + w]
                )

            def ep_den(q):
                chunks = R_CHUNKS[q]
                pd = psm.tile([128, 512], F32, tag="pp", name="pd")
                for ci, g in enumerate(chunks):
                    nc.tensor.transpose(
                        pd[:, ci : ci + 1],
                        srow[0:1, 128 * g : 128 * (g + 1)],
                        id1,
                    )
                g0 = chunks[0]
                nc.vector.reciprocal(
                    recipT[:, g0 : g0 + len(chunks)], pd[:, 0 : len(chunks)]
                )

            def ep_out(q, i):
                g = R_CHUNKS[q][i]
                pf = psm.tile([128, 512], F32, tag="pp", name="pf")
                nc.tensor.matmul(
                    pf[:], oT[:, 128 * g : 128 * (g + 1)], wo_t[:],
                    start=True, stop=True,
                )
                osb = opool.tile([128, D], F16, tag="osb")
                if g % 2 == 0:
                    nc.vector.tensor_scalar_mul(osb[:], pf[:], recipT[:, g : g + 1])
                else:
                    nc.scalar.activation(
                        osb[:], pf[:], mybir.ActivationFunctionType.Copy,
                        bias=0.0, scale=recipT[:, g : g + 1],
                    )
                eng = nc.sync if g % 2 == 0 else nc.scalar
                eng.dma_start(d_out[128 * g : 128 * (g + 1), :], osb[:])

            # ---- schedule ----
            qproj(0)
            qproj(1)
            kvproj(0)
            kvproj(1)
            for j in range(8):
                if j == 4:
                    kvproj(2)
                scores(j)
                if j >= 1:
                    pv(j - 1)
            scores(8)
            scores(9)
            pv(7)
            ep_copy(0)
            kvproj(3)
            scores(10)
            scores(11)
            pv(8)
            pv(9)
            ep_den(0)
            ep_out(0, 0)
            scores(12)
            scores(13)
            pv(10)
            pv(11)
            ep_out(0, 1)
            ep_out(0, 2)
            scores(14, 15)
            pv(12)
            pv(13)
            ep_out(0, 3)
            ep_copy(1)
            ep_den(1)
            ep_out(1, 0)
            pv(14)
            ep_out(1, 1)
            pv(15)
            ep_copy(2)
            ep_den(2)
            ep_out(2, 0)
            ep_out(1, 2)


_NC_CACHE = None


def _get_nc():
    global _NC_CACHE
    if _NC_CACHE is None:
        _NC_CACHE = _build()
    return _NC_CACHE


def make_in_maps(query, key, Wq, Wk, Wv, Wo):
    query = np.asarray(query, dtype=np.float32)
    key = np.asarray(key, dtype=np.float32)
    Wq = np.asarray(Wq, dtype=np.float32)
    Wk = np.asarray(Wk, dtype=np.float32)
    Wv = np.asarray(Wv, dtype=np.float32)
    Wo = np.asarray(Wo, dtype=np.float32)

    wo_r = Wo.reshape(H, DV, D).sum(axis=0).astype(np.float16)  # [DV, D]
    wqkv = np.concatenate([Wq, Wq, Wk, Wk, Wv, Wv], axis=1).astype(np.float16)
    cst = np.zeros((128, C_NCOL), np.float16)
    cst[:, 0:1536] = wqkv.reshape(4, 128, 384).transpose(1, 0, 2).reshape(128, 1536)
    cst[:, C_TRI : C_TRI + 128] = np.triu(np.ones((128, 128), np.float16))

    in_maps = []
    for c in range(N_CORES):
        b, p = divmod(c, 2)
        blocks = BLOCKS[p]
        rows = np.concatenate(
            [np.arange(128 * g, 128 * (g + 1)) for g in blocks]
        )
        qT = np.ascontiguousarray(query[b][rows].T).astype(np.float16)
        kT = np.ascontiguousarray(key[b].T).astype(np.float16)
        msk = np.zeros((128, 2 * NB + 1), np.float32)
        for j in range(NB):
            g = blocks[QB - WIDTHS[j]]
            if g == j:
                msk[:, j] = 1.0  # b: tri plane
            elif g > j:
                msk[:, NB + j] = 1.0  # a: all-ones plane
            # g < j: both zero -> all-zeros plane
        msk[:, 2 * NB] = 1.0  # 1x1 identity for the denom row transposes
        in_maps.append({"qT": qT, "kT": kT, "cst": cst, "wo": wo_r, "msk": msk})
    return in_maps


def gather_output(results, bias_term):
    """results: list of per-core {'out': [SQ, D]}; adds host-folded bias."""
    out = np.empty((B, S, D), np.float32)
    for c in range(N_CORES):
        b, p = divmod(c, 2)
        blocks = BLOCKS[p]
        co = np.asarray(results[c]["out"], dtype=np.float32)
        for bp, g in enumerate(blocks):
            out[b, 128 * g : 128 * (g + 1), :] = co[128 * bp : 128 * (bp + 1), :]
    out += bias_term
    return out


def kernel(query, key, value, Wq, bq, Wk, bk, Wv, bv, Wo, bo):
    from concourse import bass_utils

    nc = _get_nc()
    in_maps = make_in_maps(query, key, Wq, Wk, Wv, Wo)
    res = bass_utils.run_bass_kernel_spmd(
        nc, in_maps, core_ids=list(range(N_CORES))
    )
    Wo = np.asarray(Wo, dtype=np.float32)
    wo_r = Wo.reshape(H, DV, D).sum(axis=0)
    # bq/bk only shift scores per query row (softmax-invariant); bv adds
    # bv @ Wo_r to every output row since attention rows sum to 1.
    bias_term = np.asarray(bv, np.float32) @ wo_r + np.asarray(bo, np.float32)
    return gather_output(res.results, bias_term.astype(np.float32))
